# revision 1
# baseline (speedup 1.0000x reference)
import sys

sys.path.insert(0, "/opt/trn_rl_repo")

import numpy as np

D_MODEL = 1024
NUM_HEADS = 16
HEAD_DIM = 64
B = 2
S = 2048
N_CORES = 8
HG = 4          # head-groups (cores per batch)
HPC = 4         # heads per core
DL = 256        # local feature width per core (HPC * HEAD_DIM)

_cache = {}
last_exec_time_ns = None


def _build(has_qkvb):
    import concourse.bacc as bacc
    import concourse.mybir as mybir
    import concourse.tile as tile

    F32 = mybir.dt.float32
    F32R = mybir.dt.float32r
    Exp = mybir.ActivationFunctionType.Exp
    mult = mybir.AluOpType.mult
    is_ge = mybir.AluOpType.is_ge

    nc = bacc.Bacc("TRN2", target_bir_lowering=False, debug=False)
    xT_d = nc.dram_tensor("xT", (D_MODEL, S), F32, kind="ExternalInput")
    wq_d = nc.dram_tensor("wqkvT", (D_MODEL, 3 * DL), F32, kind="ExternalInput")
    wo_d = nc.dram_tensor("woT", (DL, D_MODEL), F32, kind="ExternalInput")
    if has_qkvb:
        qb_d = nc.dram_tensor("qb", (1, 3 * DL), F32, kind="ExternalInput")
    out_d = nc.dram_tensor("out", (S, D_MODEL), F32, kind="ExternalOutput")

    def r(ap):
        return ap.bitcast(F32R)

    with tile.TileContext(nc) as tc:
        with tc.tile_pool(name="persist", bufs=1) as persist:
            # Q/K packed per head-pair p: partitions 0:64 head 2p, 64:128 head 2p+1
            QT = [persist.tile([128, S], F32, name=f"QT{p}") for p in range(2)]
            KT = [persist.tile([128, S], F32, name=f"KT{p}") for p in range(2)]
            # V augmented: per s-tile block of 128 cols: [V dims 64 | ones 64]
            Vaug = [persist.tile([128, S], F32, name=f"Vg{h}") for h in range(HPC)]

            with tc.tile_pool(name="work", bufs=1) as work:
                with tc.tile_pool(name="projin", bufs=1) as projin, \
                     tc.tile_pool(name="pproj", bufs=1, space="PSUM") as pproj:
                    xt = [projin.tile([128, S], F32, name=f"xt{i}") for i in range(8)]
                    wq = [projin.tile([128, 3 * DL], F32, name=f"wq{i}") for i in range(8)]
                    for i in range(8):
                        nc.sync.dma_start(out=r(xt[i][:]), in_=r(xT_d[128 * i:128 * (i + 1), :]))
                        nc.sync.dma_start(out=r(wq[i][:]), in_=r(wq_d[128 * i:128 * (i + 1), :]))
                    if has_qkvb:
                        qb_t = projin.tile([1, 3 * DL], F32, name="qb_t")
                        nc.sync.dma_start(out=r(qb_t[:]), in_=r(qb_d[:]))
                        ones_t = projin.tile([1, 512], F32, name="ones_t")
                        nc.vector.memset(ones_t[:], 1.0)

                    # ---- QK projection: mi 0/1 -> QT[0/1], 2/3 -> KT[0/1]
                    for mi in range(4):
                        dst = QT[mi] if mi < 2 else KT[mi - 2]
                        for n in range(4):
                            psq = pproj.tile([128, 512], F32, tag="qk", bufs=2, name="psq")
                            for i in range(8):
                                nc.tensor.matmul(
                                    out=psq[:],
                                    lhsT=r(wq[i][:, 128 * mi:128 * (mi + 1)]),
                                    rhs=r(xt[i][:, 512 * n:512 * (n + 1)]),
                                    start=(i == 0),
                                    stop=(i == 7 and not has_qkvb),
                                )
                            if has_qkvb:
                                nc.tensor.matmul(
                                    out=psq[:],
                                    lhsT=r(qb_t[0:1, 128 * mi:128 * (mi + 1)]),
                                    rhs=r(ones_t[0:1, :]),
                                    start=False, stop=True,
                                )
                            nc.vector.tensor_copy(out=r(dst[:, 512 * n:512 * (n + 1)]), in_=psq[:])

                    # ---- V projection into Vaug (interleaved [V|ones] blocks)
                    for h in range(HPC):
                        nc.vector.memset(Vaug[h][:], 1.0)
                    for st in range(16):
                        psv = pproj.tile([128, DL], F32, tag="v", bufs=2, name="psv")
                        for i in range(8):
                            nc.tensor.matmul(
                                out=psv[:],
                                lhsT=r(xt[i][:, 128 * st:128 * (st + 1)]),
                                rhs=r(wq[i][:, 512:768]),
                                start=(i == 0),
                                stop=(i == 7 and not has_qkvb),
                            )
                        if has_qkvb:
                            nc.tensor.matmul(
                                out=psv[:],
                                lhsT=r(ones_t[0:1, 0:128]),
                                rhs=r(qb_t[0:1, 512:768]),
                                start=False, stop=True,
                            )
                        for h in range(HPC):
                            nc.vector.tensor_copy(
                                out=r(Vaug[h][:, 128 * st:128 * st + 64]),
                                in_=psv[:, 64 * h:64 * h + 64],
                            )

                with tc.tile_pool(name="persist2", bufs=1) as persist2:
                    ctxh = [persist2.tile([64, S], F32, name=f"ctx{h}") for h in range(HPC)]
                    woh = [persist2.tile([64, D_MODEL], F32, name=f"woh{h}") for h in range(HPC)]
                    for h in range(HPC):
                        nc.sync.dma_start(out=r(woh[h][:]), in_=r(wo_d[64 * h:64 * (h + 1), :]))

                    # ---- attention
                    with tc.tile_pool(name="pattn", bufs=1, space="PSUM") as pattn:
                        for p in range(2):
                            for j in range(4):
                                mlast = 4 * j + 3
                                psA = pattn.tile([128, 512], F32, tag="a", bufs=2, name="psA")
                                psB = pattn.tile([128, 512], F32, tag="b", bufs=2, name="psB")
                                for m in range(4 * j + 4):
                                    psS0 = pattn.tile([128, 512], F32, tag="s0", bufs=2, name="psS0")
                                    psS1 = pattn.tile([128, 512], F32, tag="s1", bufs=2, name="psS1")
                                    nc.tensor.matmul(
                                        out=psS0[:],
                                        lhsT=r(KT[p][0:64, 128 * m:128 * (m + 1)]),
                                        rhs=r(QT[p][0:64, 512 * j:512 * (j + 1)]),
                                        start=True, stop=True,
                                    )
                                    nc.tensor.matmul(
                                        out=psS1[:],
                                        lhsT=r(KT[p][64:128, 128 * m:128 * (m + 1)]),
                                        rhs=r(QT[p][64:128, 512 * j:512 * (j + 1)]),
                                        start=True, stop=True,
                                    )
                                    e0 = work.tile([128, 512], F32, tag="e0", bufs=3, name="e0")
                                    e1 = work.tile([128, 512], F32, tag="e1", bufs=3, name="e1")
                                    nc.scalar.activation(r(e0[:]), psS0[:], Exp, scale=0.125)
                                    nc.scalar.activation(r(e1[:]), psS1[:], Exp, scale=0.125)
                                    t = m - 4 * j
                                    if t >= 0:
                                        w = 128 * (t + 1)
                                        for e in (e0, e1):
                                            nc.gpsimd.affine_select(
                                                out=r(e[:, 0:w]), in_=r(e[:, 0:w]),
                                                pattern=[[1, w]],
                                                channel_multiplier=-1,
                                                base=-128 * t,
                                                compare_op=is_ge,
                                                fill=0.0,
                                            )
                                    nc.tensor.matmul(
                                        out=psA[:],
                                        lhsT=r(Vaug[2 * p][:, 128 * m:128 * (m + 1)]),
                                        rhs=r(e0[:]),
                                        start=(m == 0), stop=(m == mlast),
                                    )
                                    nc.tensor.matmul(
                                        out=psB[:],
                                        lhsT=r(Vaug[2 * p + 1][:, 128 * m:128 * (m + 1)]),
                                        rhs=r(e1[:]),
                                        start=(m == 0), stop=(m == mlast),
                                    )
                                # normalize: ctx[h][:, j] = psX[0:64] / sums(psX[64:128])
                                for ps, h in ((psA, 2 * p), (psB, 2 * p + 1)):
                                    sums = work.tile([64, 512], F32, tag="sums", bufs=2, name="sums")
                                    nc.vector.tensor_copy(out=sums[:], in_=ps[64:128, :])
                                    rec = work.tile([64, 512], F32, tag="rec", bufs=2, name="rec")
                                    nc.vector.reciprocal_approx_fast(rec[:], sums[:])
                                    nc.vector.tensor_tensor(
                                        out=r(ctxh[h][:, 512 * j:512 * (j + 1)]),
                                        in0=ps[0:64, :],
                                        in1=rec[:],
                                        op=mult,
                                    )

                    # ---- output projection
                    with tc.tile_pool(name="outst", bufs=1) as outst, \
                         tc.tile_pool(name="pout", bufs=1, space="PSUM") as pout:
                        for qm in range(16):
                            stage = outst.tile([128, D_MODEL], F32, tag="st", bufs=3, name="stage")
                            for n in range(2):
                                pso = pout.tile([128, 512], F32, tag=f"o{n}", bufs=2, name="pso")
                                for h in range(HPC):
                                    nc.tensor.matmul(
                                        out=pso[:],
                                        lhsT=r(ctxh[h][:, 128 * qm:128 * (qm + 1)]),
                                        rhs=r(woh[h][:, 512 * n:512 * (n + 1)]),
                                        start=(h == 0), stop=(h == HPC - 1),
                                    )
                                nc.vector.tensor_copy(out=stage[:, 512 * n:512 * (n + 1)], in_=pso[:])
                            nc.sync.dma_start(out=out_d[128 * qm:128 * (qm + 1), :], in_=stage[:])

    nc.finalize()
    return nc


def kernel(x, qkv_w, qkv_b, out_w, out_b):
    from concourse import bass_utils
    global last_exec_time_ns

    x = np.ascontiguousarray(np.asarray(x, dtype=np.float32))
    qkv_w = np.asarray(qkv_w, dtype=np.float32)
    qkv_b = np.asarray(qkv_b, dtype=np.float32)
    out_w = np.asarray(out_w, dtype=np.float32)
    out_b = np.asarray(out_b, dtype=np.float32)

    has_qkvb = bool(np.any(qkv_b))
    if has_qkvb not in _cache:
        _cache[has_qkvb] = _build(has_qkvb)
    nc = _cache[has_qkvb]

    in_maps = []
    for c in range(N_CORES):
        b, hg = divmod(c, HG)
        xT = np.ascontiguousarray(x[b].T)
        rows = np.concatenate([
            qkv_w[DL * hg:DL * (hg + 1)],
            qkv_w[D_MODEL + DL * hg:D_MODEL + DL * (hg + 1)],
            qkv_w[2 * D_MODEL + DL * hg:2 * D_MODEL + DL * (hg + 1)],
        ], axis=0)
        wqkvT = np.ascontiguousarray(rows.T)
        woT = np.ascontiguousarray(out_w[:, DL * hg:DL * (hg + 1)].T)
        m = {"xT": xT, "wqkvT": wqkvT, "woT": woT}
        if has_qkvb:
            m["qb"] = np.concatenate([
                qkv_b[DL * hg:DL * (hg + 1)],
                qkv_b[D_MODEL + DL * hg:D_MODEL + DL * (hg + 1)],
                qkv_b[2 * D_MODEL + DL * hg:2 * D_MODEL + DL * (hg + 1)],
            ]).reshape(1, 3 * DL).astype(np.float32)
        in_maps.append(m)

    res = bass_utils.run_bass_kernel_spmd(nc, in_maps, core_ids=list(range(N_CORES)))
    last_exec_time_ns = res.exec_time_ns

    out = np.zeros((B, S, D_MODEL), dtype=np.float32)
    for c in range(N_CORES):
        b, hg = divmod(c, HG)
        out[b] += res.results[c]["out"]
    out += out_b[None, None, :]
    return out


# revision 5
# speedup vs baseline: 1.0962x; 1.0962x over previous
import sys

sys.path.insert(0, "/opt/trn_rl_repo")

import numpy as np

D_MODEL = 1024
NUM_HEADS = 16
HEAD_DIM = 64
B = 2
S = 2048
N_CORES = 8
HG = 4          # head-groups (cores per batch)
HPC = 4         # heads per core
DL = 256        # local feature width per core (HPC * HEAD_DIM)

_cache = {}
last_exec_time_ns = None


def _build(has_qkvb):
    import concourse.bacc as bacc
    import concourse.mybir as mybir
    import concourse.tile as tile

    F32 = mybir.dt.float32
    F32R = mybir.dt.float32r
    Exp = mybir.ActivationFunctionType.Exp
    mult = mybir.AluOpType.mult
    is_ge = mybir.AluOpType.is_ge

    nc = bacc.Bacc("TRN2", target_bir_lowering=False, debug=False)
    xT_d = nc.dram_tensor("xT", (D_MODEL, S), F32, kind="ExternalInput")
    wq_d = nc.dram_tensor("wqkvT", (D_MODEL, 3 * DL), F32, kind="ExternalInput")
    wo_d = nc.dram_tensor("woT", (DL, D_MODEL), F32, kind="ExternalInput")
    if has_qkvb:
        qb_d = nc.dram_tensor("qb", (1, 3 * DL), F32, kind="ExternalInput")
    out_d = nc.dram_tensor("out", (S, D_MODEL), F32, kind="ExternalOutput")

    def r(ap):
        return ap.bitcast(F32R)

    with tile.TileContext(nc) as tc:
        with tc.tile_pool(name="persist", bufs=1) as persist:
            # Q/K packed per head-pair p: partitions 0:64 head 2p, 64:128 head 2p+1
            QT = [persist.tile([128, S], F32, name=f"QT{p}") for p in range(2)]
            KT = [persist.tile([128, S], F32, name=f"KT{p}") for p in range(2)]
            # V augmented: per s-tile block of 128 cols: [V dims 64 | ones 64]
            Vaug = [persist.tile([128, S], F32, name=f"Vg{h}") for h in range(HPC)]
            # prebaked causal band masks: mask[t][k, q] = 1 if q >= k + 128t else 0
            maskt = [persist.tile([128, 512], F32, name=f"mask{t}") for t in range(4)]
            for t in range(4):
                nc.vector.memset(maskt[t][:], 1.0)
                w = 128 * (t + 1)
                nc.gpsimd.affine_select(
                    out=r(maskt[t][:, 0:w]), in_=r(maskt[t][:, 0:w]),
                    pattern=[[1, w]],
                    channel_multiplier=-1,
                    base=-128 * t,
                    compare_op=is_ge,
                    fill=0.0,
                )

            with tc.tile_pool(name="work", bufs=1) as work:
                with tc.tile_pool(name="projin", bufs=1) as projin, \
                     tc.tile_pool(name="pproj", bufs=1, space="PSUM") as pproj:
                    xt = [projin.tile([128, S], F32, name=f"xt{i}") for i in range(8)]
                    wq = [projin.tile([128, 3 * DL], F32, name=f"wq{i}") for i in range(8)]
                    for i in range(8):
                        nc.sync.dma_start(out=r(xt[i][:]), in_=r(xT_d[128 * i:128 * (i + 1), :]))
                        nc.sync.dma_start(out=r(wq[i][:]), in_=r(wq_d[128 * i:128 * (i + 1), :]))
                    if has_qkvb:
                        qb_t = projin.tile([1, 3 * DL], F32, name="qb_t")
                        nc.sync.dma_start(out=r(qb_t[:]), in_=r(qb_d[:]))
                        ones_t = projin.tile([1, 512], F32, name="ones_t")
                        nc.vector.memset(ones_t[:], 1.0)

                    # ---- QK projection: mi 0/1 -> QT[0/1], 2/3 -> KT[0/1]
                    for mi in range(4):
                        dst = QT[mi] if mi < 2 else KT[mi - 2]
                        for n in range(4):
                            psq = pproj.tile([128, 512], F32, tag="qk", bufs=2, name="psq")
                            for i in range(8):
                                nc.tensor.matmul(
                                    out=psq[:],
                                    lhsT=r(wq[i][:, 128 * mi:128 * (mi + 1)]),
                                    rhs=r(xt[i][:, 512 * n:512 * (n + 1)]),
                                    start=(i == 0),
                                    stop=(i == 7 and not has_qkvb),
                                )
                            if has_qkvb:
                                nc.tensor.matmul(
                                    out=psq[:],
                                    lhsT=r(qb_t[0:1, 128 * mi:128 * (mi + 1)]),
                                    rhs=r(ones_t[0:1, :]),
                                    start=False, stop=True,
                                )
                            nc.vector.tensor_copy(out=r(dst[:, 512 * n:512 * (n + 1)]), in_=psq[:])

                    # ---- V projection into Vaug (interleaved [V|ones] blocks)
                    for h in range(HPC):
                        nc.vector.memset(Vaug[h][:], 1.0)
                    for st in range(16):
                        psv = pproj.tile([128, DL], F32, tag="v", bufs=2, name="psv")
                        for i in range(8):
                            nc.tensor.matmul(
                                out=psv[:],
                                lhsT=r(xt[i][:, 128 * st:128 * (st + 1)]),
                                rhs=r(wq[i][:, 512:768]),
                                start=(i == 0),
                                stop=(i == 7 and not has_qkvb),
                            )
                        if has_qkvb:
                            nc.tensor.matmul(
                                out=psv[:],
                                lhsT=r(ones_t[0:1, 0:128]),
                                rhs=r(qb_t[0:1, 512:768]),
                                start=False, stop=True,
                            )
                        for h in range(HPC):
                            nc.vector.tensor_copy(
                                out=r(Vaug[h][:, 128 * st:128 * st + 64]),
                                in_=psv[:, 64 * h:64 * h + 64],
                            )

                with tc.tile_pool(name="persist2", bufs=1) as persist2:
                    # ctx pair-packed: head 2p at partitions 0:64, head 2p+1 at 64:128
                    ctxp = [persist2.tile([128, S], F32, name=f"ctxp{p}") for p in range(2)]
                    wop = [persist2.tile([128, D_MODEL], F32, name=f"wop{p}") for p in range(2)]
                    for p in range(2):
                        nc.sync.dma_start(out=r(wop[p][:]), in_=r(wo_d[128 * p:128 * (p + 1), :]))

                    # ---- attention
                    with tc.tile_pool(name="pattn", bufs=1, space="PSUM") as pattn:
                        for p in range(2):
                            for j in range(4):
                                mlast = 4 * j + 3
                                psA = pattn.tile([128, 512], F32, tag="a", bufs=2, name="psA")
                                psB = pattn.tile([128, 512], F32, tag="b", bufs=2, name="psB")
                                for m in range(4 * j + 4):
                                    psS0 = pattn.tile([128, 512], F32, tag="s0", bufs=2, name="psS0")
                                    psS1 = pattn.tile([128, 512], F32, tag="s1", bufs=2, name="psS1")
                                    nc.tensor.matmul(
                                        out=psS0[:],
                                        lhsT=r(KT[p][0:64, 128 * m:128 * (m + 1)]),
                                        rhs=r(QT[p][0:64, 512 * j:512 * (j + 1)]),
                                        start=True, stop=True,
                                    )
                                    nc.tensor.matmul(
                                        out=psS1[:],
                                        lhsT=r(KT[p][64:128, 128 * m:128 * (m + 1)]),
                                        rhs=r(QT[p][64:128, 512 * j:512 * (j + 1)]),
                                        start=True, stop=True,
                                    )
                                    e0 = work.tile([128, 512], F32, tag="e0", bufs=3, name="e0")
                                    e1 = work.tile([128, 512], F32, tag="e1", bufs=3, name="e1")
                                    t = m - 4 * j
                                    if t >= 0:
                                        # band tile: exp then out-of-place DVE mask multiply
                                        f0 = work.tile([128, 512], F32, tag="f0", bufs=2, name="f0")
                                        f1 = work.tile([128, 512], F32, tag="f1", bufs=2, name="f1")
                                        nc.scalar.activation(r(f0[:]), psS0[:], Exp, scale=0.125)
                                        nc.scalar.activation(r(f1[:]), psS1[:], Exp, scale=0.125)
                                        nc.vector.tensor_tensor(
                                            out=r(e0[:]), in0=f0[:], in1=maskt[t][:], op=mult)
                                        nc.vector.tensor_tensor(
                                            out=r(e1[:]), in0=f1[:], in1=maskt[t][:], op=mult)
                                    else:
                                        nc.scalar.activation(r(e0[:]), psS0[:], Exp, scale=0.125)
                                        nc.scalar.activation(r(e1[:]), psS1[:], Exp, scale=0.125)
                                    nc.tensor.matmul(
                                        out=psA[:],
                                        lhsT=r(Vaug[2 * p][:, 128 * m:128 * (m + 1)]),
                                        rhs=r(e0[:]),
                                        start=(m == 0), stop=(m == mlast),
                                    )
                                    nc.tensor.matmul(
                                        out=psB[:],
                                        lhsT=r(Vaug[2 * p + 1][:, 128 * m:128 * (m + 1)]),
                                        rhs=r(e1[:]),
                                        start=(m == 0), stop=(m == mlast),
                                    )
                                # normalize: ctxp[p][0:64|64:128, j] = psX[0:64] / sums
                                sums = work.tile([64, 512], F32, tag="sums", bufs=2, name="sums")
                                nc.vector.tensor_copy(out=sums[:], in_=psA[64:128, :])
                                rec = work.tile([64, 512], F32, tag="rec", bufs=2, name="rec")
                                nc.vector.reciprocal_approx_fast(rec[:], sums[:])
                                nc.vector.tensor_tensor(
                                    out=r(ctxp[p][0:64, 512 * j:512 * (j + 1)]),
                                    in0=psA[0:64, :],
                                    in1=rec[:],
                                    op=mult,
                                )
                                sums2 = work.tile([64, 512], F32, tag="sums", bufs=2, name="sums")
                                nc.vector.tensor_copy(out=sums2[:], in_=psB[64:128, :])
                                rec2 = work.tile([64, 512], F32, tag="rec", bufs=2, name="rec")
                                nc.vector.reciprocal_approx_fast(rec2[:], sums2[:])
                                codd = work.tile([64, 512], F32, tag="codd", bufs=2, name="codd")
                                nc.vector.tensor_tensor(
                                    out=codd[:], in0=psB[0:64, :], in1=rec2[:], op=mult)
                                nc.vector.tensor_copy(
                                    out=r(ctxp[p][64:128, 512 * j:512 * (j + 1)]), in_=codd[:])

                    # ---- output projection
                    with tc.tile_pool(name="outst", bufs=1) as outst, \
                         tc.tile_pool(name="pout", bufs=1, space="PSUM") as pout:
                        for qm in range(16):
                            stage = outst.tile([128, D_MODEL], F32, tag="st", bufs=3, name="stage")
                            for n in range(2):
                                pso = pout.tile([128, 512], F32, tag=f"o{n}", bufs=2, name="pso")
                                for p in range(2):
                                    nc.tensor.matmul(
                                        out=pso[:],
                                        lhsT=r(ctxp[p][:, 128 * qm:128 * (qm + 1)]),
                                        rhs=r(wop[p][:, 512 * n:512 * (n + 1)]),
                                        start=(p == 0), stop=(p == 1),
                                    )
                                nc.vector.tensor_copy(out=stage[:, 512 * n:512 * (n + 1)], in_=pso[:])
                            nc.sync.dma_start(out=out_d[128 * qm:128 * (qm + 1), :], in_=stage[:])

    nc.finalize()
    return nc


def kernel(x, qkv_w, qkv_b, out_w, out_b):
    from concourse import bass_utils
    global last_exec_time_ns

    x = np.ascontiguousarray(np.asarray(x, dtype=np.float32))
    qkv_w = np.asarray(qkv_w, dtype=np.float32)
    qkv_b = np.asarray(qkv_b, dtype=np.float32)
    out_w = np.asarray(out_w, dtype=np.float32)
    out_b = np.asarray(out_b, dtype=np.float32)

    has_qkvb = bool(np.any(qkv_b))
    if has_qkvb not in _cache:
        _cache[has_qkvb] = _build(has_qkvb)
    nc = _cache[has_qkvb]

    in_maps = []
    for c in range(N_CORES):
        b, hg = divmod(c, HG)
        xT = np.ascontiguousarray(x[b].T)
        rows = np.concatenate([
            qkv_w[DL * hg:DL * (hg + 1)],
            qkv_w[D_MODEL + DL * hg:D_MODEL + DL * (hg + 1)],
            qkv_w[2 * D_MODEL + DL * hg:2 * D_MODEL + DL * (hg + 1)],
        ], axis=0)
        wqkvT = np.ascontiguousarray(rows.T)
        woT = np.ascontiguousarray(out_w[:, DL * hg:DL * (hg + 1)].T)
        m = {"xT": xT, "wqkvT": wqkvT, "woT": woT}
        if has_qkvb:
            m["qb"] = np.concatenate([
                qkv_b[DL * hg:DL * (hg + 1)],
                qkv_b[D_MODEL + DL * hg:D_MODEL + DL * (hg + 1)],
                qkv_b[2 * D_MODEL + DL * hg:2 * D_MODEL + DL * (hg + 1)],
            ]).reshape(1, 3 * DL).astype(np.float32)
        in_maps.append(m)

    res = bass_utils.run_bass_kernel_spmd(nc, in_maps, core_ids=list(range(N_CORES)))
    last_exec_time_ns = res.exec_time_ns

    out = np.zeros((B, S, D_MODEL), dtype=np.float32)
    for c in range(N_CORES):
        b, hg = divmod(c, HG)
        out[b] += res.results[c]["out"]
    out += out_b[None, None, :]
    return out
